# revision 38
# baseline (speedup 1.0000x reference)
"""4-layer GCN (GCNConv+ReLU x4, 128-64-32-64-128) on 8 Trainium2 NeuronCores.

Strategy (dst-sharded message passing):
  - Host: deg/norm precompute. out[d] = s[d]*(sum_{src->d} h'[src] + h'[d]) + b
    with h' = (s*x) @ W and s = deg^-1/2 -- so no per-edge scaling is needed.
  - dst nodes sharded across 8 cores (12500 each), degree-sorted into tiles
    of 128. Edges sorted by (dst tile, src quarter), padded to 128-multiples
    per (tile, quarter) with zero-row indices; per-(tile,quarter) column
    counts are maxed across cores so all cores run one SPMD program.
  - Device, per layer: h' = x~^T @ W (PE, node-major tiles, bf16) -> one
    batched shard store -> AllGather table in DRAM -> dma_gather 256B rows
    per edge (int16 local indices into 4 quarter windows) -> one-hot M
    matrices (DVE is_equal in 2x_1p mode: [e, d, col] layout vs a
    replicated iota, one op per dst tile) -> PE matmul segment-sum
    accumulated in PSUM (+ identity-matmul self term) -> ReLU/scale.
  - Gathers round-robin the 4 SWDGE queues (queue q runs on Q7 cores
    2q/2q+1, so 4 gathers generate descriptors concurrently; queue =
    emission index % 4 keeps each of Tile's 8 DMASW sem lanes single-queue
    so per-lane completion counting stays FIFO -- asserted post-compile).
  - M builds are emitted one tile ahead of the previous tile's matmuls so
    epilogue PSUM->SBUF copies never head-of-line block the DVE.
  - Layers 1-3 keep x~ feature-major in SBUF; layer 4 emits node-major
    output tiles, DMA'd out; host inverse-permutes.
"""

import numpy as np

# ---------------------------------------------------------------------------
# configuration
# ---------------------------------------------------------------------------

P = 128
FPAD = 128          # table row elements (bf16 -> 256B rows)
NCORES = 8
GROUP_TILES = 5     # dst tiles per gather group
MB = 8              # M-matrix build batch (chunks per DVE op)


class Cfg:
    def __init__(self, n_nodes, channels=(128, 64, 32, 64, 128)):
        self.N = n_nodes
        self.NPC = n_nodes // NCORES
        self.NTILES = (self.NPC + P - 1) // P
        self.NPAD = self.NTILES * P
        self.SHARD_ROWS = self.NPC + 1          # + zero row
        self.QWIN = 2 * self.SHARD_ROWS
        self.ZERO_LOCAL = self.NPC
        self.channels = channels
        self.dims = list(zip(channels[:-1], channels[1:]))


FULL = Cfg(100000)

# ---------------------------------------------------------------------------
# host preprocessing
# ---------------------------------------------------------------------------


def preprocess(edge_index, cfg: Cfg):
    src = np.asarray(edge_index[0], dtype=np.int64)
    dst = np.asarray(edge_index[1], dtype=np.int64)
    N, NPC, NTILES = cfg.N, cfg.NPC, cfg.NTILES

    deg = np.bincount(dst, minlength=N).astype(np.float32) + 1.0
    deg_isqrt = (1.0 / np.sqrt(deg)).astype(np.float32)

    core_of_node = np.minimum(np.arange(N) // NPC, NCORES - 1)
    node_order = np.empty((NCORES, cfg.NPAD), dtype=np.int64)
    node_valid = np.zeros((NCORES, cfg.NPAD), dtype=bool)
    rank_of = np.empty(N, dtype=np.int64)
    for c in range(NCORES):
        nodes = np.arange(c * NPC, (c + 1) * NPC)
        perm = nodes[np.argsort(deg[nodes], kind="stable")]
        node_order[c, :NPC] = perm
        node_order[c, NPC:] = perm[-1] if NPC else 0
        node_valid[c, :NPC] = True
        rank_of[perm] = np.arange(NPC)

    src_core = src // NPC
    src_quarter = src_core // 2
    src_local = (src_core % 2) * cfg.SHARD_ROWS + rank_of[src]

    dst_core = dst // NPC
    dst_rank = rank_of[dst]
    dst_tile = dst_rank // P
    dst_slot = dst_rank % P

    key = (dst_core * NTILES + dst_tile) * 4 + src_quarter
    order = np.argsort(key, kind="stable")
    key_s = key[order]
    src_local_s = src_local[order].astype(np.int32)
    dst_slot_s = dst_slot[order].astype(np.int32)

    counts = np.bincount(key_s, minlength=NCORES * NTILES * 4)
    counts = counts.reshape(NCORES, NTILES, 4)
    kcols = np.ceil(counts.max(axis=0) / P).astype(np.int64)    # [NTILES, 4]

    # Global column order: group-major, then quarter, then tile-in-group.
    # Each (group, quarter) gather segment is a contiguous column range of
    # the group's msgs tile.
    ngroups = (NTILES + GROUP_TILES - 1) // GROUP_TILES
    col_off = np.zeros((NTILES, 4), dtype=np.int64)
    group_col0 = np.zeros(ngroups + 1, dtype=np.int64)   # group col ranges
    segs = []                                            # (g, q, col0, ncols)
    acc = 0
    for g in range(ngroups):
        group_col0[g] = acc
        t0, t1 = g * GROUP_TILES, min((g + 1) * GROUP_TILES, NTILES)
        for q in range(4):
            c0 = acc
            for t in range(t0, t1):
                col_off[t, q] = acc
                acc += int(kcols[t, q])
            if acc > c0:
                segs.append((g, q, int(c0), int(acc - c0)))
    group_col0[ngroups] = acc
    total_cols = acc
    stream_len = total_cols * P

    idx16 = np.full((NCORES, stream_len), cfg.ZERO_LOCAL, dtype=np.int32)
    dstloc = np.zeros((NCORES, stream_len), dtype=np.int32)
    starts = np.zeros(NCORES * NTILES * 4 + 1, dtype=np.int64)
    np.cumsum(np.bincount(key_s, minlength=NCORES * NTILES * 4),
              out=starts[1:])
    for c in range(NCORES):
        base = c * NTILES * 4
        for t in range(NTILES):
            for q in range(4):
                k = base + t * 4 + q
                a, b = starts[k], starts[k + 1]
                if b > a:
                    pos0 = col_off[t, q] * P
                    idx16[c, pos0:pos0 + (b - a)] = src_local_s[a:b]
                    dstloc[c, pos0:pos0 + (b - a)] = dst_slot_s[a:b]
    assert idx16.max(initial=0) <= 32767

    # tile-major column order (for dstloc / M builds): for t, for q.
    # tm_of[global_col] = tile-major position; tile_tm0[t] = first tm pos.
    tm_of = np.zeros(total_cols, dtype=np.int64)
    tile_tm0 = np.zeros(NTILES + 1, dtype=np.int64)
    tm = 0
    for t in range(NTILES):
        tile_tm0[t] = tm
        for q in range(4):
            o = int(col_off[t, q])
            for j in range(int(kcols[t, q])):
                tm_of[o + j] = tm
                tm += 1
    tile_tm0[NTILES] = tm
    assert tm == total_cols

    maxc = int((tile_tm0[1:] - tile_tm0[:-1]).max())

    meta = dict(
        cfg=cfg, kcols=kcols, col_off=col_off, total_cols=total_cols,
        stream_len=stream_len, node_order=node_order, node_valid=node_valid,
        deg_isqrt=deg_isqrt, ngroups=ngroups, segs=segs, group_col0=group_col0,
        tm_of=tm_of, tile_tm0=tile_tm0, maxc=maxc,
    )
    return idx16, dstloc, meta

# ---------------------------------------------------------------------------
# device program
# ---------------------------------------------------------------------------


def build_program(meta, with_bias, debug_stop=None):
    import concourse.bass as bass
    import concourse.bacc as bacc
    import concourse.tile as tile
    from concourse import mybir

    cfg: Cfg = meta["cfg"]
    NT, NPAD = cfg.NTILES, cfg.NPAD
    kcols, col_off = meta["kcols"], meta["col_off"]
    total_cols, stream_len = meta["total_cols"], meta["stream_len"]
    ngroups, segs, group_col0 = meta["ngroups"], meta["segs"], meta["group_col0"]
    tm_of, tile_tm0 = meta["tm_of"], meta["tile_tm0"]
    MAXC = meta["maxc"]
    dims = cfg.dims
    NL = len(dims)
    f32, bf16, i16 = mybir.dt.float32, mybir.dt.bfloat16, mybir.dt.int16
    AF = mybir.ActivationFunctionType
    OP = mybir.AluOpType

    nc = bacc.Bacc("TRN2", target_bir_lowering=False, debug=False,
                   num_devices=NCORES, num_swdge_queues=4)

    # ---- I/O ----
    xT_d = nc.dram_tensor("xT", [dims[0][0], NPAD], bf16, kind="ExternalInput")
    idx_d = nc.dram_tensor("idx16", [P, stream_len // 16], i16,
                           kind="ExternalInput")
    dstloc_d = nc.dram_tensor("dstloc", [P, total_cols], bf16,
                              kind="ExternalInput")
    scol2_d = nc.dram_tensor("scol2", [P, NT], f32, kind="ExternalInput")
    scol1_d = nc.dram_tensor("scol1", [P, NT], f32, kind="ExternalInput")
    iota_d = nc.dram_tensor("iota_rep", [P, P * MAXC], bf16,
                            kind="ExternalInput")
    ident_d = nc.dram_tensor("ident", [P, P], bf16, kind="ExternalInput")
    W_d = [nc.dram_tensor(f"W{l+1}", [dims[l][0], FPAD], bf16,
                          kind="ExternalInput") for l in range(NL)]
    sbias_d = [nc.dram_tensor(f"sbias{l+1}", [NPAD, dims[l][1]], f32,
                              kind="ExternalInput") if with_bias else None
               for l in range(NL)]
    out_d = nc.dram_tensor("out", [NPAD, dims[-1][1]], f32,
                           kind="ExternalOutput")

    shard_d = [nc.dram_tensor(f"shard{l}", [cfg.SHARD_ROWS, FPAD], bf16)
               for l in range(NL)]
    # Shared addr_space enables the fast HBM-HBM AllGather path (the
    # compiler warns that non-Shared outputs take a slow bounce).
    table_d = [nc.dram_tensor(f"table{l}", [NCORES * cfg.SHARD_ROWS, FPAD],
                              bf16, addr_space="Shared") for l in range(NL)]

    with tile.TileContext(nc) as tc:
        import contextlib
        ctx = contextlib.ExitStack()
        with ctx:
            pers = ctx.enter_context(tc.tile_pool(name="pers", bufs=1))
            msgs_pool = ctx.enter_context(tc.tile_pool(name="msgs", bufs=3))
            idx_pool = ctx.enter_context(tc.tile_pool(name="idxp", bufs=2))
            m_pool = ctx.enter_context(tc.tile_pool(name="mmat", bufs=3))
            tmp_pool = ctx.enter_context(tc.tile_pool(name="tmp", bufs=4))
            psum_h = ctx.enter_context(
                tc.tile_pool(name="psum_h", bufs=2, space="PSUM"))
            psum_o = ctx.enter_context(
                tc.tile_pool(name="psum_o", bufs=4, space="PSUM"))

            # ---- persistent SBUF ----
            xT = pers.tile([P, NPAD], bf16, tag="xT")
            dstloc_sb = pers.tile([P, total_cols], bf16, tag="dstloc")
            scol2 = pers.tile([P, NT], f32, tag="scol2")
            scol1 = pers.tile([P, NT], f32, tag="scol1")
            iota = pers.tile([P, P * MAXC], bf16, tag="iota")
            ident = pers.tile([P, P], bf16, tag="ident")
            hnode = pers.tile([P, NT * FPAD], bf16, tag="hnode")
            zrow = pers.tile([1, FPAD], bf16, tag="zrow")
            W_sb = [pers.tile([dims[l][0], FPAD], bf16, tag=f"W{l}",
                              name=f"Wsb{l}") for l in range(NL)]


            nc.sync.dma_start(xT[:], xT_d[:, :])
            nc.sync.dma_start(dstloc_sb[:], dstloc_d[:, :])
            nc.sync.dma_start(scol2[:], scol2_d[:, :])
            nc.sync.dma_start(scol1[:], scol1_d[:, :])
            nc.sync.dma_start(iota[:], iota_d[:, :])
            nc.sync.dma_start(ident[:], ident_d[:, :])
            for l in range(NL):
                nc.sync.dma_start(W_sb[l][:], W_d[l][:, :])
            nc.gpsimd.memset(zrow[:], 0.0)

            gather_emit_count = [0]

            def emit_shard(lx):
                # batched shard store: DRAM row t*128+p <- hnode[p, t, :]
                nc.sync.dma_start(
                    shard_d[lx][0:(NT - 1) * P, :].rearrange(
                        "(t p) f -> p t f", p=P),
                    hnode[:, 0:(NT - 1) * FPAD].rearrange(
                        "p (t f) -> p t f", f=FPAD))
                tail = cfg.NPC - (NT - 1) * P
                nc.sync.dma_start(
                    shard_d[lx][(NT - 1) * P:cfg.NPC, :],
                    hnode[0:tail, (NT - 1) * FPAD:NT * FPAD])
                nc.sync.dma_start(shard_d[lx][cfg.NPC:cfg.NPC + 1, :],
                                  zrow[:, :])
                nc.gpsimd.collective_compute(
                    "AllGather", OP.bypass,
                    replica_groups=[list(range(NCORES))],
                    ins=[shard_d[lx][:, :]],
                    outs=[table_d[lx][:, :]],
                )

            def emit_compute(l, t, mt, cols_t, ttm0, m3, gc0):
                Fi, Fo = dims[l]
                last_layer = l == NL - 1
                cols = []
                for q in range(4):
                    o = int(col_off[t, q])
                    cols += list(range(o, o + int(kcols[t, q])))
                mt3 = mt[:, 0:cols_t * P].rearrange("p (d j) -> p d j",
                                                    j=cols_t)
                po = psum_o.tile([P, Fo], f32, tag="po")
                for j, c in enumerate(cols):
                    jj = int(tm_of[c]) - ttm0
                    assert 0 <= jj < cols_t
                    nc.tensor.matmul(
                        po[:], lhsT=mt3[:, :, jj],
                        rhs=m3[:, c - gc0, 0:Fo], start=j == 0,
                        stop=False)
                hsl = hnode[:, t * FPAD:t * FPAD + Fo]
                nc.tensor.matmul(po[:], lhsT=ident[:], rhs=hsl,
                                 start=len(cols) == 0, stop=True)

                # ---- epilogue (node-major) ----
                csl = slice(t * P, (t + 1) * P)
                scol = scol2 if not last_layer else scol1
                src = po[:]
                if with_bias:
                    sb_t = tmp_pool.tile([P, Fo], f32, tag="sbias")
                    nc.sync.dma_start(
                        sb_t[:], sbias_d[l][t * P:(t + 1) * P, :])
                    t1b = tmp_pool.tile([P, Fo], f32, tag="tmpb")
                    nc.vector.scalar_tensor_tensor(
                        t1b[:], po[:], scol[:, t:t + 1], sb_t[:],
                        op0=OP.mult, op1=OP.add)
                    src = t1b[:]
                akw = {} if with_bias else {"scale": scol[:, t:t + 1]}
                if not last_layer:
                    ot = tmp_pool.tile([P, Fo], bf16, tag="otile")
                    nc.scalar.activation(ot[:], src, AF.Relu, **akw)
                    pt = psum_h.tile([Fo, P], bf16, tag="ptr")
                    nc.tensor.transpose(pt[:], ot[:], ident[:])
                    nc.vector.tensor_copy(xT[0:Fo, csl], pt[:])
                else:
                    ot = tmp_pool.tile([P, Fo], f32, tag="otilef")
                    nc.scalar.activation(ot[:], src, AF.Relu, **akw)
                    nc.sync.dma_start(out_d[t * P:(t + 1) * P, :], ot[:])

            iota3 = iota[:].rearrange("p (d j) -> p d j", j=MAXC)

            group_state = {}

            def emit_gather_group(l, g):
                gc0, gc1 = int(group_col0[g]), int(group_col0[g + 1])
                gcols = gc1 - gc0
                msgs = msgs_pool.tile([P, gcols * FPAD], bf16, tag="msgs")
                m3 = msgs[:].rearrange("p (k f) -> p k f", f=FPAD)
                idxg = idx_pool.tile([P, gcols * 8], i16, tag="idxg")
                nc.sync.dma_start(idxg[:], idx_d[:, gc0 * 8:gc1 * 8])
                group_state[g] = m3
                for (sg, sq, c0, ncols) in segs:
                    if sg != g:
                        continue
                    nidx = ncols * P
                    # queue = emission index % 4 so each DMASW sem lane
                    # (8, round-robin) only ever sees one SWDGE queue --
                    # keeps per-lane completion counting monotonic.
                    qn = gather_emit_count[0] % 4
                    gather_emit_count[0] += 1
                    nc.gpsimd.dma_gather(
                        m3[:, c0 - gc0:c0 - gc0 + ncols, :],
                        table_d[l][sq * cfg.QWIN:(sq + 1) * cfg.QWIN, :],
                        idxg[:, (c0 - gc0) * 8:(c0 - gc0 + ncols) * 8],
                        nidx, nidx, FPAD,
                        single_packet=nidx <= 1024,
                        queue_num=qn,
                    )

            for l in range(NL):
                Fi, Fo = dims[l]
                last_layer = l == NL - 1

                # ---- phase 1: h' = x~^T @ W, node-major bf16 tiles ----
                for t in range(NT):
                    csl = slice(t * P, (t + 1) * P)
                    ph = psum_h.tile([P, FPAD], f32, tag="ph")
                    nc.tensor.matmul(ph[:], lhsT=xT[0:Fi, csl],
                                     rhs=W_sb[l][:, :], start=True, stop=True)
                    nc.vector.tensor_copy(hnode[:, t * FPAD:(t + 1) * FPAD],
                                          ph[:])
                emit_shard(l)

                # gather + segment-sum, software-pipelined: the M build for
                # tile t is emitted one tile ahead of tile t-1's matmuls so
                # the DVE never head-of-line blocks PE.
                group_state = {}
                pending = None

                def compute_group(g, l=l):
                    nonlocal pending
                    t0 = g * GROUP_TILES
                    t1 = min(t0 + GROUP_TILES, NT)
                    gc0 = int(group_col0[g])
                    m3 = group_state[g]
                    for t in range(t0, t1):
                        ttm0, ttm1 = int(tile_tm0[t]), int(tile_tm0[t + 1])
                        cols_t = ttm1 - ttm0
                        mt = m_pool.tile([P, MAXC * P], bf16, tag="m")
                        mt3 = mt[:, 0:cols_t * P].rearrange(
                            "p (d j) -> p d j", j=cols_t)
                        din = dstloc_sb[:, ttm0:ttm1]
                        din3 = din[:, None, :].to_broadcast([P, P, cols_t])
                        nc.vector.tensor_tensor(mt3[:], din3,
                                                iota3[:, :, 0:cols_t],
                                                op=OP.is_equal)
                        if pending is not None:
                            emit_compute(**pending)
                        pending = dict(l=l, t=t, mt=mt, cols_t=cols_t,
                                       ttm0=ttm0, m3=m3, gc0=gc0)

                lead = min(2, ngroups)
                for g in range(lead):
                    emit_gather_group(l, g)
                for g in range(lead, ngroups):
                    compute_group(g - lead)
                    emit_gather_group(l, g)
                for g in range(max(0, ngroups - lead), ngroups):
                    compute_group(g)
                if pending is not None:
                    emit_compute(**pending)
                    pending = None

    nc.compile()
    # Verify the DMASW-lane/queue alignment assumption: Pool-engine DMA
    # instructions must be exactly the gathers, in emission order, with
    # queue_num cycling 0..3 (8 lanes % 4 queues == 0 keeps each lane
    # single-queue, so per-lane DMA completion stays FIFO).
    gathers = [
        inst
        for block in nc.main_func.blocks
        for inst in block.instructions
        if isinstance(inst, mybir.InstDMAGatherAnt)
    ]
    assert all(g.queue_num == i % 4 for i, g in enumerate(gathers)), \
        [g.queue_num for g in gathers[:12]]
    return nc

# ---------------------------------------------------------------------------
# runtime glue
# ---------------------------------------------------------------------------


def _bf16(a):
    import ml_dtypes
    return np.asarray(a, dtype=np.float32).astype(ml_dtypes.bfloat16)


def build_inputs(x, Ws, bs, idx16, dstloc, meta, with_bias):
    cfg: Cfg = meta["cfg"]
    node_order, node_valid = meta["node_order"], meta["node_valid"]
    deg_isqrt = meta["deg_isqrt"]
    segs = meta["segs"]
    x = np.asarray(x, dtype=np.float32)

    # wrap idx streams per gather segment ([16, L/16]), replicated to 128
    # partitions (8 copies: 4 queues x 2 Q7 cores read their own band)
    idxw = np.zeros((NCORES, 16, meta["stream_len"] // 16), dtype=np.int16)
    for c in range(NCORES):
        for (_g, _q, c0, ncols) in segs:
            seg = idx16[c, c0 * P:(c0 + ncols) * P]
            idxw[c, :, c0 * 8:(c0 + ncols) * 8] = (
                seg.reshape(-1, 16).T.astype(np.int16))
    idxw = np.tile(idxw, (1, 8, 1))

    dl = dstloc.reshape(NCORES, meta["total_cols"], P)
    dl_tm = np.zeros_like(dl)
    dl_tm[:, meta["tm_of"], :] = dl          # tile-major column order
    dstloc_dev = _bf16(np.transpose(dl_tm, (0, 2, 1)))  # [NC, 128, cols]

    maxc = meta["maxc"]
    # iota_rep[p, d*MAXC + j] = d (constant along j) -- dense innermost axis
    # for the 2x-mode M build.
    iota_rep = _bf16(np.broadcast_to(
        np.repeat(np.arange(P, dtype=np.float32), maxc)[None, :],
        (P, P * maxc)))
    ident = _bf16(np.eye(P, dtype=np.float32))

    in_maps = []
    for c in range(NCORES):
        nodes = node_order[c]
        valid = node_valid[c]
        s1 = np.where(valid, deg_isqrt[nodes], 0.0).astype(np.float32)
        xt = (x[nodes] * s1[:, None]).astype(np.float32)     # [NPAD, Fin]
        m = {
            "xT": _bf16(np.ascontiguousarray(xt.T)),         # [Fin, NPAD]
            "idx16": idxw[c],
            "dstloc": dstloc_dev[c],
            "scol2": np.ascontiguousarray(
                (s1 * s1).reshape(cfg.NTILES, P).T).astype(np.float32),
            "scol1": np.ascontiguousarray(
                s1.reshape(cfg.NTILES, P).T).astype(np.float32),
            "iota_rep": iota_rep,
            "ident": ident,
        }
        for l in range(len(Ws)):
            W = np.asarray(Ws[l], dtype=np.float32)
            Wp = np.zeros((W.shape[0], FPAD), dtype=np.float32)
            Wp[:, :W.shape[1]] = W
            m[f"W{l+1}"] = _bf16(Wp)
            if with_bias:
                b = np.asarray(bs[l], dtype=np.float32)
                if l < len(Ws) - 1:
                    m[f"sbias{l+1}"] = np.ascontiguousarray(
                        s1[:, None] * b[None, :]).astype(np.float32)
                else:
                    m[f"sbias{l+1}"] = np.ascontiguousarray(
                        np.broadcast_to(b[None, :], (cfg.NPAD, len(b)))
                    ).astype(np.float32)
        in_maps.append(m)
    return in_maps


def assemble_output(results, meta, n_out_feats):
    cfg: Cfg = meta["cfg"]
    node_order, node_valid = meta["node_order"], meta["node_valid"]
    full = np.zeros((cfg.N, n_out_feats), dtype=np.float32)
    for c in range(NCORES):
        out_c = np.asarray(results[c]["out"], dtype=np.float32)
        full[node_order[c, :cfg.NPC]] = out_c[:cfg.NPC]
    return full


_PROGRAM_CACHE = {}


def run(x, edge_index, Ws, bs, cfg):
    from concourse.bass_utils import run_bass_kernel_spmd

    idx16, dstloc, meta = preprocess(edge_index, cfg)
    with_bias = any(np.any(np.asarray(b)) for b in bs)

    key = (cfg.N, tuple(cfg.channels), meta["total_cols"],
           tuple(np.asarray(meta["kcols"]).ravel()), with_bias)
    if key not in _PROGRAM_CACHE:
        _PROGRAM_CACHE[key] = build_program(meta, with_bias)
    nc = _PROGRAM_CACHE[key]

    in_maps = build_inputs(x, Ws, bs, idx16, dstloc, meta, with_bias)
    res = run_bass_kernel_spmd(nc, in_maps, list(range(NCORES)))
    return assemble_output(res.results, meta, cfg.channels[-1])


def kernel(x, edge_index, W1, b1, W2, b2, W3, b3, W4, b4):
    return run(x, edge_index, [W1, W2, W3, W4], [b1, b2, b3, b4], FULL)



# revision 39
# speedup vs baseline: 1.0220x; 1.0220x over previous
"""4-layer GCN (GCNConv+ReLU x4, 128-64-32-64-128) on 8 Trainium2 NeuronCores.

Strategy (dst-sharded message passing):
  - Host: deg/norm precompute. out[d] = s[d]*(sum_{src->d} h'[src] + h'[d]) + b
    with h' = (s*x) @ W and s = deg^-1/2 -- so no per-edge scaling is needed.
  - dst nodes sharded across 8 cores (12500 each), degree-sorted into tiles
    of 128. Edges sorted by (dst tile, src quarter), padded to 128-multiples
    per (tile, quarter) with zero-row indices; per-(tile,quarter) column
    counts are maxed across cores so all cores run one SPMD program.
  - Device, per layer: h' = x~^T @ W (PE, node-major tiles, bf16) -> one
    batched shard store -> AllGather table in DRAM -> dma_gather 256B rows
    per edge (int16 local indices into 4 quarter windows) -> one-hot M
    matrices (DVE is_equal in 2x_1p mode: [e, d, col] layout vs a
    replicated iota, one op per dst tile) -> PE matmul segment-sum
    accumulated in PSUM (+ identity-matmul self term) -> ReLU/scale.
  - Gathers round-robin the 4 SWDGE queues (queue q runs on Q7 cores
    2q/2q+1, so 4 gathers generate descriptors concurrently; queue =
    emission index % 4 keeps each of Tile's 8 DMASW sem lanes single-queue
    so per-lane completion counting stays FIFO -- asserted post-compile).
  - M builds are emitted one tile ahead of the previous tile's matmuls so
    epilogue PSUM->SBUF copies never head-of-line block the DVE.
  - Layers 1-3 keep x~ feature-major in SBUF; layer 4 emits node-major
    output tiles, DMA'd out; host inverse-permutes.
"""

import numpy as np

# ---------------------------------------------------------------------------
# configuration
# ---------------------------------------------------------------------------

P = 128
FPAD = 128          # table row elements (bf16 -> 256B rows)
NCORES = 8
GROUP_TILES = 7     # dst tiles per gather group
MB = 8              # M-matrix build batch (chunks per DVE op)


class Cfg:
    def __init__(self, n_nodes, channels=(128, 64, 32, 64, 128)):
        self.N = n_nodes
        self.NPC = n_nodes // NCORES
        self.NTILES = (self.NPC + P - 1) // P
        self.NPAD = self.NTILES * P
        self.SHARD_ROWS = self.NPC + 1          # + zero row
        self.QWIN = 2 * self.SHARD_ROWS
        self.ZERO_LOCAL = self.NPC
        self.channels = channels
        self.dims = list(zip(channels[:-1], channels[1:]))


FULL = Cfg(100000)

# ---------------------------------------------------------------------------
# host preprocessing
# ---------------------------------------------------------------------------


def preprocess(edge_index, cfg: Cfg):
    src = np.asarray(edge_index[0], dtype=np.int64)
    dst = np.asarray(edge_index[1], dtype=np.int64)
    N, NPC, NTILES = cfg.N, cfg.NPC, cfg.NTILES

    deg = np.bincount(dst, minlength=N).astype(np.float32) + 1.0
    deg_isqrt = (1.0 / np.sqrt(deg)).astype(np.float32)

    core_of_node = np.minimum(np.arange(N) // NPC, NCORES - 1)
    node_order = np.empty((NCORES, cfg.NPAD), dtype=np.int64)
    node_valid = np.zeros((NCORES, cfg.NPAD), dtype=bool)
    rank_of = np.empty(N, dtype=np.int64)
    for c in range(NCORES):
        nodes = np.arange(c * NPC, (c + 1) * NPC)
        perm = nodes[np.argsort(deg[nodes], kind="stable")]
        node_order[c, :NPC] = perm
        node_order[c, NPC:] = perm[-1] if NPC else 0
        node_valid[c, :NPC] = True
        rank_of[perm] = np.arange(NPC)

    src_core = src // NPC
    src_quarter = src_core // 2
    src_local = (src_core % 2) * cfg.SHARD_ROWS + rank_of[src]

    dst_core = dst // NPC
    dst_rank = rank_of[dst]
    dst_tile = dst_rank // P
    dst_slot = dst_rank % P

    key = (dst_core * NTILES + dst_tile) * 4 + src_quarter
    order = np.argsort(key, kind="stable")
    key_s = key[order]
    src_local_s = src_local[order].astype(np.int32)
    dst_slot_s = dst_slot[order].astype(np.int32)

    counts = np.bincount(key_s, minlength=NCORES * NTILES * 4)
    counts = counts.reshape(NCORES, NTILES, 4)
    kcols = np.ceil(counts.max(axis=0) / P).astype(np.int64)    # [NTILES, 4]

    # Global column order: group-major, then quarter, then tile-in-group.
    # Each (group, quarter) gather segment is a contiguous column range of
    # the group's msgs tile.
    ngroups = (NTILES + GROUP_TILES - 1) // GROUP_TILES
    col_off = np.zeros((NTILES, 4), dtype=np.int64)
    group_col0 = np.zeros(ngroups + 1, dtype=np.int64)   # group col ranges
    segs = []                                            # (g, q, col0, ncols)
    acc = 0
    for g in range(ngroups):
        group_col0[g] = acc
        t0, t1 = g * GROUP_TILES, min((g + 1) * GROUP_TILES, NTILES)
        for q in range(4):
            c0 = acc
            for t in range(t0, t1):
                col_off[t, q] = acc
                acc += int(kcols[t, q])
            if acc > c0:
                segs.append((g, q, int(c0), int(acc - c0)))
    group_col0[ngroups] = acc
    total_cols = acc
    stream_len = total_cols * P

    idx16 = np.full((NCORES, stream_len), cfg.ZERO_LOCAL, dtype=np.int32)
    dstloc = np.zeros((NCORES, stream_len), dtype=np.int32)
    starts = np.zeros(NCORES * NTILES * 4 + 1, dtype=np.int64)
    np.cumsum(np.bincount(key_s, minlength=NCORES * NTILES * 4),
              out=starts[1:])
    for c in range(NCORES):
        base = c * NTILES * 4
        for t in range(NTILES):
            for q in range(4):
                k = base + t * 4 + q
                a, b = starts[k], starts[k + 1]
                if b > a:
                    pos0 = col_off[t, q] * P
                    idx16[c, pos0:pos0 + (b - a)] = src_local_s[a:b]
                    dstloc[c, pos0:pos0 + (b - a)] = dst_slot_s[a:b]
    assert idx16.max(initial=0) <= 32767

    # tile-major column order (for dstloc / M builds): for t, for q.
    # tm_of[global_col] = tile-major position; tile_tm0[t] = first tm pos.
    tm_of = np.zeros(total_cols, dtype=np.int64)
    tile_tm0 = np.zeros(NTILES + 1, dtype=np.int64)
    tm = 0
    for t in range(NTILES):
        tile_tm0[t] = tm
        for q in range(4):
            o = int(col_off[t, q])
            for j in range(int(kcols[t, q])):
                tm_of[o + j] = tm
                tm += 1
    tile_tm0[NTILES] = tm
    assert tm == total_cols

    maxc = int((tile_tm0[1:] - tile_tm0[:-1]).max())

    meta = dict(
        cfg=cfg, kcols=kcols, col_off=col_off, total_cols=total_cols,
        stream_len=stream_len, node_order=node_order, node_valid=node_valid,
        deg_isqrt=deg_isqrt, ngroups=ngroups, segs=segs, group_col0=group_col0,
        tm_of=tm_of, tile_tm0=tile_tm0, maxc=maxc,
    )
    return idx16, dstloc, meta

# ---------------------------------------------------------------------------
# device program
# ---------------------------------------------------------------------------


def build_program(meta, with_bias, debug_stop=None):
    import concourse.bass as bass
    import concourse.bacc as bacc
    import concourse.tile as tile
    from concourse import mybir

    cfg: Cfg = meta["cfg"]
    NT, NPAD = cfg.NTILES, cfg.NPAD
    kcols, col_off = meta["kcols"], meta["col_off"]
    total_cols, stream_len = meta["total_cols"], meta["stream_len"]
    ngroups, segs, group_col0 = meta["ngroups"], meta["segs"], meta["group_col0"]
    tm_of, tile_tm0 = meta["tm_of"], meta["tile_tm0"]
    MAXC = meta["maxc"]
    dims = cfg.dims
    NL = len(dims)
    f32, bf16, i16 = mybir.dt.float32, mybir.dt.bfloat16, mybir.dt.int16
    AF = mybir.ActivationFunctionType
    OP = mybir.AluOpType

    nc = bacc.Bacc("TRN2", target_bir_lowering=False, debug=False,
                   num_devices=NCORES, num_swdge_queues=4)

    # ---- I/O ----
    xT_d = nc.dram_tensor("xT", [dims[0][0], NPAD], bf16, kind="ExternalInput")
    idx_d = nc.dram_tensor("idx16", [P, stream_len // 16], i16,
                           kind="ExternalInput")
    dstloc_d = nc.dram_tensor("dstloc", [P, total_cols], bf16,
                              kind="ExternalInput")
    scol2_d = nc.dram_tensor("scol2", [P, NT], f32, kind="ExternalInput")
    scol1_d = nc.dram_tensor("scol1", [P, NT], f32, kind="ExternalInput")
    iota_d = nc.dram_tensor("iota_rep", [P, P * MAXC], bf16,
                            kind="ExternalInput")
    ident_d = nc.dram_tensor("ident", [P, P], bf16, kind="ExternalInput")
    W_d = [nc.dram_tensor(f"W{l+1}", [dims[l][0], FPAD], bf16,
                          kind="ExternalInput") for l in range(NL)]
    sbias_d = [nc.dram_tensor(f"sbias{l+1}", [NPAD, dims[l][1]], f32,
                              kind="ExternalInput") if with_bias else None
               for l in range(NL)]
    out_d = nc.dram_tensor("out", [NPAD, dims[-1][1]], f32,
                           kind="ExternalOutput")

    shard_d = [nc.dram_tensor(f"shard{l}", [cfg.SHARD_ROWS, FPAD], bf16)
               for l in range(NL)]
    # Shared addr_space enables the fast HBM-HBM AllGather path (the
    # compiler warns that non-Shared outputs take a slow bounce).
    table_d = [nc.dram_tensor(f"table{l}", [NCORES * cfg.SHARD_ROWS, FPAD],
                              bf16, addr_space="Shared") for l in range(NL)]

    with tile.TileContext(nc) as tc:
        import contextlib
        ctx = contextlib.ExitStack()
        with ctx:
            pers = ctx.enter_context(tc.tile_pool(name="pers", bufs=1))
            msgs_pool = ctx.enter_context(tc.tile_pool(name="msgs", bufs=2))
            idx_pool = ctx.enter_context(tc.tile_pool(name="idxp", bufs=2))
            m_pool = ctx.enter_context(tc.tile_pool(name="mmat", bufs=3))
            tmp_pool = ctx.enter_context(tc.tile_pool(name="tmp", bufs=4))
            psum_h = ctx.enter_context(
                tc.tile_pool(name="psum_h", bufs=2, space="PSUM"))
            psum_o = ctx.enter_context(
                tc.tile_pool(name="psum_o", bufs=4, space="PSUM"))

            # ---- persistent SBUF ----
            xT = pers.tile([P, NPAD], bf16, tag="xT")
            dstloc_sb = pers.tile([P, total_cols], bf16, tag="dstloc")
            scol2 = pers.tile([P, NT], f32, tag="scol2")
            scol1 = pers.tile([P, NT], f32, tag="scol1")
            iota = pers.tile([P, P * MAXC], bf16, tag="iota")
            ident = pers.tile([P, P], bf16, tag="ident")
            hnode = pers.tile([P, NT * FPAD], bf16, tag="hnode")
            zrow = pers.tile([1, FPAD], bf16, tag="zrow")
            W_sb = [pers.tile([dims[l][0], FPAD], bf16, tag=f"W{l}",
                              name=f"Wsb{l}") for l in range(NL)]


            nc.sync.dma_start(xT[:], xT_d[:, :])
            nc.sync.dma_start(dstloc_sb[:], dstloc_d[:, :])
            nc.sync.dma_start(scol2[:], scol2_d[:, :])
            nc.sync.dma_start(scol1[:], scol1_d[:, :])
            nc.sync.dma_start(iota[:], iota_d[:, :])
            nc.sync.dma_start(ident[:], ident_d[:, :])
            for l in range(NL):
                nc.sync.dma_start(W_sb[l][:], W_d[l][:, :])
            nc.gpsimd.memset(zrow[:], 0.0)

            gather_emit_count = [0]

            def emit_shard(lx):
                # batched shard store: DRAM row t*128+p <- hnode[p, t, :]
                nc.sync.dma_start(
                    shard_d[lx][0:(NT - 1) * P, :].rearrange(
                        "(t p) f -> p t f", p=P),
                    hnode[:, 0:(NT - 1) * FPAD].rearrange(
                        "p (t f) -> p t f", f=FPAD))
                tail = cfg.NPC - (NT - 1) * P
                nc.sync.dma_start(
                    shard_d[lx][(NT - 1) * P:cfg.NPC, :],
                    hnode[0:tail, (NT - 1) * FPAD:NT * FPAD])
                nc.sync.dma_start(shard_d[lx][cfg.NPC:cfg.NPC + 1, :],
                                  zrow[:, :])
                nc.gpsimd.collective_compute(
                    "AllGather", OP.bypass,
                    replica_groups=[list(range(NCORES))],
                    ins=[shard_d[lx][:, :]],
                    outs=[table_d[lx][:, :]],
                )

            def emit_compute(l, t, mt, cols_t, ttm0, m3, gc0):
                Fi, Fo = dims[l]
                last_layer = l == NL - 1
                cols = []
                for q in range(4):
                    o = int(col_off[t, q])
                    cols += list(range(o, o + int(kcols[t, q])))
                mt3 = mt[:, 0:cols_t * P].rearrange("p (d j) -> p d j",
                                                    j=cols_t)
                po = psum_o.tile([P, Fo], f32, tag="po")
                for j, c in enumerate(cols):
                    jj = int(tm_of[c]) - ttm0
                    assert 0 <= jj < cols_t
                    nc.tensor.matmul(
                        po[:], lhsT=mt3[:, :, jj],
                        rhs=m3[:, c - gc0, 0:Fo], start=j == 0,
                        stop=False)
                hsl = hnode[:, t * FPAD:t * FPAD + Fo]
                nc.tensor.matmul(po[:], lhsT=ident[:], rhs=hsl,
                                 start=len(cols) == 0, stop=True)

                # ---- epilogue (node-major) ----
                csl = slice(t * P, (t + 1) * P)
                scol = scol2 if not last_layer else scol1
                src = po[:]
                if with_bias:
                    sb_t = tmp_pool.tile([P, Fo], f32, tag="sbias")
                    nc.sync.dma_start(
                        sb_t[:], sbias_d[l][t * P:(t + 1) * P, :])
                    t1b = tmp_pool.tile([P, Fo], f32, tag="tmpb")
                    nc.vector.scalar_tensor_tensor(
                        t1b[:], po[:], scol[:, t:t + 1], sb_t[:],
                        op0=OP.mult, op1=OP.add)
                    src = t1b[:]
                akw = {} if with_bias else {"scale": scol[:, t:t + 1]}
                if not last_layer:
                    ot = tmp_pool.tile([P, Fo], bf16, tag="otile")
                    nc.scalar.activation(ot[:], src, AF.Relu, **akw)
                    pt = psum_h.tile([Fo, P], bf16, tag="ptr")
                    nc.tensor.transpose(pt[:], ot[:], ident[:])
                    nc.vector.tensor_copy(xT[0:Fo, csl], pt[:])
                else:
                    ot = tmp_pool.tile([P, Fo], f32, tag="otilef")
                    nc.scalar.activation(ot[:], src, AF.Relu, **akw)
                    nc.sync.dma_start(out_d[t * P:(t + 1) * P, :], ot[:])

            iota3 = iota[:].rearrange("p (d j) -> p d j", j=MAXC)

            group_state = {}

            def emit_gather_group(l, g):
                gc0, gc1 = int(group_col0[g]), int(group_col0[g + 1])
                gcols = gc1 - gc0
                msgs = msgs_pool.tile([P, gcols * FPAD], bf16, tag="msgs")
                m3 = msgs[:].rearrange("p (k f) -> p k f", f=FPAD)
                idxg = idx_pool.tile([P, gcols * 8], i16, tag="idxg")
                nc.sync.dma_start(idxg[:], idx_d[:, gc0 * 8:gc1 * 8])
                group_state[g] = m3
                for (sg, sq, c0, ncols) in segs:
                    if sg != g:
                        continue
                    nidx = ncols * P
                    # queue = emission index % 4 so each DMASW sem lane
                    # (8, round-robin) only ever sees one SWDGE queue --
                    # keeps per-lane completion counting monotonic.
                    qn = gather_emit_count[0] % 4
                    gather_emit_count[0] += 1
                    nc.gpsimd.dma_gather(
                        m3[:, c0 - gc0:c0 - gc0 + ncols, :],
                        table_d[l][sq * cfg.QWIN:(sq + 1) * cfg.QWIN, :],
                        idxg[:, (c0 - gc0) * 8:(c0 - gc0 + ncols) * 8],
                        nidx, nidx, FPAD,
                        single_packet=nidx <= 1024,
                        queue_num=qn,
                    )

            for l in range(NL):
                Fi, Fo = dims[l]
                last_layer = l == NL - 1

                # ---- phase 1: h' = x~^T @ W, node-major bf16 tiles ----
                for t in range(NT):
                    csl = slice(t * P, (t + 1) * P)
                    ph = psum_h.tile([P, FPAD], f32, tag="ph")
                    nc.tensor.matmul(ph[:], lhsT=xT[0:Fi, csl],
                                     rhs=W_sb[l][:, :], start=True, stop=True)
                    nc.vector.tensor_copy(hnode[:, t * FPAD:(t + 1) * FPAD],
                                          ph[:])
                emit_shard(l)

                # gather + segment-sum, software-pipelined: the M build for
                # tile t is emitted one tile ahead of tile t-1's matmuls so
                # the DVE never head-of-line blocks PE.
                group_state = {}
                pending = None

                def compute_group(g, l=l):
                    nonlocal pending
                    t0 = g * GROUP_TILES
                    t1 = min(t0 + GROUP_TILES, NT)
                    gc0 = int(group_col0[g])
                    m3 = group_state[g]
                    for t in range(t0, t1):
                        ttm0, ttm1 = int(tile_tm0[t]), int(tile_tm0[t + 1])
                        cols_t = ttm1 - ttm0
                        mt = m_pool.tile([P, MAXC * P], bf16, tag="m")
                        mt3 = mt[:, 0:cols_t * P].rearrange(
                            "p (d j) -> p d j", j=cols_t)
                        din = dstloc_sb[:, ttm0:ttm1]
                        din3 = din[:, None, :].to_broadcast([P, P, cols_t])
                        nc.vector.tensor_tensor(mt3[:], din3,
                                                iota3[:, :, 0:cols_t],
                                                op=OP.is_equal)
                        if pending is not None:
                            emit_compute(**pending)
                        pending = dict(l=l, t=t, mt=mt, cols_t=cols_t,
                                       ttm0=ttm0, m3=m3, gc0=gc0)

                lead = min(1, ngroups)
                for g in range(lead):
                    emit_gather_group(l, g)
                for g in range(lead, ngroups):
                    compute_group(g - lead)
                    emit_gather_group(l, g)
                for g in range(max(0, ngroups - lead), ngroups):
                    compute_group(g)
                if pending is not None:
                    emit_compute(**pending)
                    pending = None

    nc.compile()
    # Verify the DMASW-lane/queue alignment assumption: Pool-engine DMA
    # instructions must be exactly the gathers, in emission order, with
    # queue_num cycling 0..3 (8 lanes % 4 queues == 0 keeps each lane
    # single-queue, so per-lane DMA completion stays FIFO).
    gathers = [
        inst
        for block in nc.main_func.blocks
        for inst in block.instructions
        if isinstance(inst, mybir.InstDMAGatherAnt)
    ]
    assert all(g.queue_num == i % 4 for i, g in enumerate(gathers)), \
        [g.queue_num for g in gathers[:12]]
    return nc

# ---------------------------------------------------------------------------
# runtime glue
# ---------------------------------------------------------------------------


def _bf16(a):
    import ml_dtypes
    return np.asarray(a, dtype=np.float32).astype(ml_dtypes.bfloat16)


def build_inputs(x, Ws, bs, idx16, dstloc, meta, with_bias):
    cfg: Cfg = meta["cfg"]
    node_order, node_valid = meta["node_order"], meta["node_valid"]
    deg_isqrt = meta["deg_isqrt"]
    segs = meta["segs"]
    x = np.asarray(x, dtype=np.float32)

    # wrap idx streams per gather segment ([16, L/16]), replicated to 128
    # partitions (8 copies: 4 queues x 2 Q7 cores read their own band)
    idxw = np.zeros((NCORES, 16, meta["stream_len"] // 16), dtype=np.int16)
    for c in range(NCORES):
        for (_g, _q, c0, ncols) in segs:
            seg = idx16[c, c0 * P:(c0 + ncols) * P]
            idxw[c, :, c0 * 8:(c0 + ncols) * 8] = (
                seg.reshape(-1, 16).T.astype(np.int16))
    idxw = np.tile(idxw, (1, 8, 1))

    dl = dstloc.reshape(NCORES, meta["total_cols"], P)
    dl_tm = np.zeros_like(dl)
    dl_tm[:, meta["tm_of"], :] = dl          # tile-major column order
    dstloc_dev = _bf16(np.transpose(dl_tm, (0, 2, 1)))  # [NC, 128, cols]

    maxc = meta["maxc"]
    # iota_rep[p, d*MAXC + j] = d (constant along j) -- dense innermost axis
    # for the 2x-mode M build.
    iota_rep = _bf16(np.broadcast_to(
        np.repeat(np.arange(P, dtype=np.float32), maxc)[None, :],
        (P, P * maxc)))
    ident = _bf16(np.eye(P, dtype=np.float32))

    in_maps = []
    for c in range(NCORES):
        nodes = node_order[c]
        valid = node_valid[c]
        s1 = np.where(valid, deg_isqrt[nodes], 0.0).astype(np.float32)
        xt = (x[nodes] * s1[:, None]).astype(np.float32)     # [NPAD, Fin]
        m = {
            "xT": _bf16(np.ascontiguousarray(xt.T)),         # [Fin, NPAD]
            "idx16": idxw[c],
            "dstloc": dstloc_dev[c],
            "scol2": np.ascontiguousarray(
                (s1 * s1).reshape(cfg.NTILES, P).T).astype(np.float32),
            "scol1": np.ascontiguousarray(
                s1.reshape(cfg.NTILES, P).T).astype(np.float32),
            "iota_rep": iota_rep,
            "ident": ident,
        }
        for l in range(len(Ws)):
            W = np.asarray(Ws[l], dtype=np.float32)
            Wp = np.zeros((W.shape[0], FPAD), dtype=np.float32)
            Wp[:, :W.shape[1]] = W
            m[f"W{l+1}"] = _bf16(Wp)
            if with_bias:
                b = np.asarray(bs[l], dtype=np.float32)
                if l < len(Ws) - 1:
                    m[f"sbias{l+1}"] = np.ascontiguousarray(
                        s1[:, None] * b[None, :]).astype(np.float32)
                else:
                    m[f"sbias{l+1}"] = np.ascontiguousarray(
                        np.broadcast_to(b[None, :], (cfg.NPAD, len(b)))
                    ).astype(np.float32)
        in_maps.append(m)
    return in_maps


def assemble_output(results, meta, n_out_feats):
    cfg: Cfg = meta["cfg"]
    node_order, node_valid = meta["node_order"], meta["node_valid"]
    full = np.zeros((cfg.N, n_out_feats), dtype=np.float32)
    for c in range(NCORES):
        out_c = np.asarray(results[c]["out"], dtype=np.float32)
        full[node_order[c, :cfg.NPC]] = out_c[:cfg.NPC]
    return full


_PROGRAM_CACHE = {}


def run(x, edge_index, Ws, bs, cfg):
    from concourse.bass_utils import run_bass_kernel_spmd

    idx16, dstloc, meta = preprocess(edge_index, cfg)
    with_bias = any(np.any(np.asarray(b)) for b in bs)

    key = (cfg.N, tuple(cfg.channels), meta["total_cols"],
           tuple(np.asarray(meta["kcols"]).ravel()), with_bias)
    if key not in _PROGRAM_CACHE:
        _PROGRAM_CACHE[key] = build_program(meta, with_bias)
    nc = _PROGRAM_CACHE[key]

    in_maps = build_inputs(x, Ws, bs, idx16, dstloc, meta, with_bias)
    res = run_bass_kernel_spmd(nc, in_maps, list(range(NCORES)))
    return assemble_output(res.results, meta, cfg.channels[-1])


def kernel(x, edge_index, W1, b1, W2, b2, W3, b3, W4, b4):
    return run(x, edge_index, [W1, W2, W3, W4], [b1, b2, b3, b4], FULL)

